# revision 16
# baseline (speedup 1.0000x reference)
"""CAML (conv + label-wise attention) kernel for Trainium2, 8 NeuronCores.

Strategy: data-parallel over batch (16 samples -> 2 per core). Everything is
fused on-chip per sample:
  gather(embed) -> PE-transpose -> conv1d(K=3, as 3 accumulated matmuls)
  -> relu -> scores = U @ h (as (L,Y)-layout matmuls) -> exp (no max-sub;
  scores are tiny by construction) -> m/denominator via one matmul with a
  ones-column -> per-label dot with fc rows -> logits. Softmax normalization
  is folded into the final scale (logit = (fc . mU) / denom + b).
Loss (scalar cross-entropy over 16x1000 logits) is computed on host.
"""

import os
import numpy as np

import concourse.bass as bass
import concourse.bacc as bacc
from concourse import mybir
from concourse.bass_utils import run_bass_kernel_spmd
from concourse.masks import make_identity
from concourse.tile import TileContext

V, E, F, K, Y = 50000, 128, 256, 3, 1000
B, L = 16, 4096
YP = 1024          # labels padded to 8*128
NCORES = 8
SPC = B // NCORES  # samples per core
NLT = L // 128     # 32 l-tiles per sample
NG = 4             # gather chunks per sample
GW = NLT // NG     # l-tiles per gather chunk (8)

F32 = mybir.dt.float32

_CACHE = {}


def _build(dt_mm):
    debug = os.environ.get("CAML_DEBUG", "0") == "1"
    nc = bacc.Bacc()

    x_d = nc.declare_dram_parameter("x", [SPC, 128, NLT], mybir.dt.int32, isOutput=False)
    emb_d = nc.declare_dram_parameter("embed", [V, E], F32, isOutput=False)
    wk_d = nc.declare_dram_parameter("convw", [K, 128, F], dt_mm, isOutput=False)
    cb_d = nc.declare_dram_parameter("convb", [128, 2], F32, isOutput=False)
    u_d = nc.declare_dram_parameter("uT", [2, 128, YP], dt_mm, isOutput=False)
    fcw_d = nc.declare_dram_parameter("fcw", [8, 128, F], F32, isOutput=False)
    fcb_d = nc.declare_dram_parameter("fcb", [128, 8], F32, isOutput=False)
    out_d = nc.declare_dram_parameter("logit", [SPC, 8, 128], F32, isOutput=True)
    if debug:
        dbg_h0 = nc.declare_dram_parameter("dbg_h0", [128, L + 2], F32, isOutput=True)
        dbg_hfl = nc.declare_dram_parameter("dbg_hfl", [128, 2, L], F32, isOutput=True)
        dbg_exps = nc.declare_dram_parameter("dbg_exps", [128, NLT, 512], F32, isOutput=True)
        dbg_pm = nc.declare_dram_parameter("dbg_pm", [128, F + 1], F32, isOutput=True)
        dbg_ht = nc.declare_dram_parameter("dbg_ht", [128, NLT, F + 1], F32, isOutput=True)

    with TileContext(nc) as tc:
        with (
            tc.tile_pool(name="singles", bufs=1) as singles,
            tc.tile_pool(name="p_emb", bufs=6) as p_emb,
            tc.tile_pool(name="p_idx", bufs=2) as p_idx,
            tc.tile_pool(name="p_epi", bufs=4) as p_epi,
            tc.tile_pool(name="p_out", bufs=1) as p_out,
            tc.tile_pool(name="ps_big", bufs=3, space="PSUM") as ps_big,
            tc.tile_pool(name="ps_t", bufs=2, space="PSUM") as ps_t,
            tc.tile_pool(name="ps_m", bufs=2, space="PSUM") as ps_m,
            tc.tile_pool(name="ps_o", bufs=1, space="PSUM") as ps_o,
        ):
            # ---- constants / weights (loaded once) ----
            ident32 = singles.tile([128, 128], F32)
            make_identity(nc, ident32[:])
            if dt_mm != F32:
                ident_dt = singles.tile([128, 128], dt_mm)
                make_identity(nc, ident_dt[:])
            else:
                ident_dt = ident32

            wk_sb = singles.tile([128, K, F], dt_mm)
            for k in range(K):
                nc.sync.dma_start(out=wk_sb[:, k, :], in_=wk_d[k])
            cb_sb = singles.tile([128, 2], F32)
            nc.sync.dma_start(out=cb_sb[:], in_=cb_d[:])
            u_sb = singles.tile([128, 2, YP], dt_mm)
            for c in range(2):
                nc.sync.dma_start(out=u_sb[:, c, :], in_=u_d[c])
            fcw_sb = singles.tile([128, 8, F], F32)
            for t in range(8):
                nc.sync.dma_start(out=fcw_sb[:, t, :], in_=fcw_d[t])
            fcb_sb = singles.tile([128, 8], F32)
            nc.sync.dma_start(out=fcb_sb[:], in_=fcb_d[:])

            # ---- persistent per-sample workspaces ----
            h0 = singles.tile([128, L + 2], dt_mm)        # (E, Lpadded)
            nc.vector.memset(h0[:, 0:1], 0.0)
            nc.vector.memset(h0[:, L + 1 : L + 2], 0.0)
            h_fl = singles.tile([128, 2, L], dt_mm)       # (F-part, fc, L)
            ht = singles.tile([128, NLT, F + 1], dt_mm)   # (L-part, lt, F | ones)
            nc.vector.memset(ht[:, :, F : F + 1], 1.0)
            exps = singles.tile([128, NLT, 512], dt_mm)   # (L-part, lt, y-chunk)
            logit_sb = singles.tile([128, SPC, 8], F32)

            for s in range(1 if debug else SPC):
                idx = p_idx.tile([128, NLT], mybir.dt.int32)
                nc.sync.dma_start(out=idx[:], in_=x_d[s])

                # ---- embedding gather + transpose into (E, L) ----
                for g in range(NG):
                    embc = p_emb.tile([128, GW * 128], F32, tag="embc")
                    for j in range(GW):
                        jj = g * GW + j
                        nc.gpsimd.indirect_dma_start(
                            out=embc[:, j * 128 : (j + 1) * 128],
                            out_offset=None,
                            in_=emb_d[:],
                            in_offset=bass.IndirectOffsetOnAxis(
                                ap=idx[:, jj : jj + 1], axis=0
                            ),
                        )
                    for j in range(GW):
                        pt = ps_t.tile([128, 128], F32, tag="pt")
                        nc.tensor.transpose(
                            out=pt[:], in_=embc[:, j * 128 : (j + 1) * 128],
                            identity=ident32[:],
                        )
                        col = 1 + g * GW * 128 + j * 128
                        nc.vector.tensor_copy(out=h0[:, col : col + 128], in_=pt[:])

                if debug and s == 0:
                    nc.gpsimd.dma_start(out=dbg_h0[:], in_=h0[:])

                # ---- conv1d + relu -> h_fl (F, L) ----
                for fc in range(2):
                    for lc in range(8):
                        pc = ps_big.tile([128, 512], F32, tag="big")
                        for k in range(K):
                            nc.tensor.matmul(
                                out=pc[:],
                                lhsT=wk_sb[:, k, fc * 128 : fc * 128 + 128],
                                rhs=h0[:, lc * 512 + k : lc * 512 + k + 512],
                                start=(k == 0),
                                stop=(k == K - 1),
                            )
                        nc.scalar.activation(
                            out=h_fl[:, fc, lc * 512 : (lc + 1) * 512],
                            in_=pc[:],
                            func=mybir.ActivationFunctionType.Relu,
                            bias=cb_sb[:, fc : fc + 1],
                        )

                if debug and s == 0:
                    nc.gpsimd.dma_start(out=dbg_hfl[:], in_=h_fl[:])

                # ---- h^T (L, F) via PE transposes ----
                for lt in range(NLT):
                    for fc in range(2):
                        pt2 = ps_t.tile([128, 128], dt_mm, tag="pt")
                        nc.tensor.transpose(
                            out=pt2[:],
                            in_=h_fl[:, fc, lt * 128 : (lt + 1) * 128],
                            identity=ident_dt[:],
                        )
                        nc.vector.tensor_copy(
                            out=ht[:, lt, fc * 128 : fc * 128 + 128], in_=pt2[:]
                        )

                if debug and s == 0:
                    nc.gpsimd.dma_start(out=dbg_ht[:], in_=ht[:])

                # ---- attention, y-chunks of 512 ----
                for yc in range(2):
                    for lt in range(NLT):
                        psc = ps_big.tile([128, 512], F32, tag="big")
                        for fc in range(2):
                            nc.tensor.matmul(
                                out=psc[:],
                                lhsT=h_fl[:, fc, lt * 128 : (lt + 1) * 128],
                                rhs=u_sb[:, fc, yc * 512 : (yc + 1) * 512],
                                start=(fc == 0),
                                stop=(fc == 1),
                            )
                        nc.scalar.activation(
                            out=exps[:, lt, :], in_=psc[:],
                            func=mybir.ActivationFunctionType.Exp,
                        )
                    if debug and s == 0 and yc == 0:
                        nc.gpsimd.dma_start(out=dbg_exps[:], in_=exps[:])
                    for ys in range(4):
                        yt = yc * 4 + ys
                        pm = ps_m.tile([128, F + 1], F32, tag="pm")
                        for lt in range(NLT):
                            nc.tensor.matmul(
                                out=pm[:],
                                lhsT=exps[:, lt, ys * 128 : (ys + 1) * 128],
                                rhs=ht[:, lt, :],
                                start=(lt == 0),
                                stop=(lt == NLT - 1),
                            )
                        if debug and s == 0 and yt == 0:
                            pmst = p_epi.tile([128, F + 1], F32, tag="pmdump")
                            nc.vector.tensor_copy(out=pmst[:], in_=pm[:])
                            nc.gpsimd.dma_start(out=dbg_pm[:], in_=pmst[:])
                        # logit[y] = (fc_w[y] . mU[y]) / denom[y] + fc_b[y]
                        prod = p_epi.tile([128, F], F32, tag="prod")
                        nc.vector.tensor_mul(prod[:], pm[:, 0:F], fcw_sb[:, yt, :])
                        ssum = p_epi.tile([128, 1], F32, tag="ssum")
                        nc.vector.reduce_sum(
                            out=ssum[:], in_=prod[:], axis=mybir.AxisListType.X
                        )
                        rcp = p_epi.tile([128, 1], F32, tag="rcp")
                        nc.vector.reciprocal(out=rcp[:], in_=pm[:, F : F + 1])
                        tl = p_epi.tile([128, 1], F32, tag="tl")
                        nc.vector.tensor_mul(tl[:], ssum[:], rcp[:])
                        nc.vector.tensor_add(
                            logit_sb[:, s, yt : yt + 1], tl[:], fcb_sb[:, yt : yt + 1]
                        )

            # ---- transpose logits to (s*t, p) rows and store ----
            po = ps_o.tile([SPC * 8, 128], F32)
            nc.tensor.transpose(
                out=po[:],
                in_=logit_sb[:].rearrange("p s t -> p (s t)"),
                identity=ident32[:],
            )
            lout = p_out.tile([SPC * 8, 128], F32)
            nc.vector.tensor_copy(out=lout[:], in_=po[:])
            nc.sync.dma_start(
                out=out_d[:].rearrange("s t p -> (s t) p"), in_=lout[:]
            )

    nc.compile()
    return nc


def _get_nc():
    dt_name = os.environ.get("CAML_DT", "bf16")
    if dt_name not in _CACHE:
        dt_mm = {"bf16": mybir.dt.bfloat16, "f32": F32,
                 "f32r": mybir.dt.float32r}[dt_name]
        _CACHE[dt_name] = (_build(dt_mm), dt_mm)
    return _CACHE[dt_name]


def _prep(dt_mm, x, embed_w, conv_w, conv_b, U_w, fc_w, fc_b):
    npdt = np.float32 if dt_mm in (F32, mybir.dt.float32r) else mybir.dt.np(dt_mm)
    wkT = np.ascontiguousarray(conv_w.transpose(2, 1, 0)).astype(npdt)  # (K,E,F)
    cb = np.ascontiguousarray(conv_b.reshape(2, 128).T).astype(np.float32)
    u_pad = np.zeros((YP, F), np.float32)
    u_pad[:Y] = U_w
    uT = np.ascontiguousarray(u_pad.T.reshape(2, 128, YP)).astype(npdt)
    fc_pad = np.zeros((YP, F), np.float32)
    fc_pad[:Y] = fc_w
    fcw = np.ascontiguousarray(fc_pad.reshape(8, 128, F))
    fcb_pad = np.zeros((YP,), np.float32)
    fcb_pad[:Y] = fc_b
    fcb = np.ascontiguousarray(fcb_pad.reshape(8, 128).T)
    common = {
        "embed": np.ascontiguousarray(embed_w, dtype=np.float32),
        "convw": wkT, "convb": cb, "uT": uT, "fcw": fcw, "fcb": fcb,
    }
    in_maps = []
    for c in range(NCORES):
        xs = x[c * SPC : (c + 1) * SPC].astype(np.int32)
        x_dev = np.ascontiguousarray(xs.reshape(SPC, NLT, 128).transpose(0, 2, 1))
        in_maps.append({"x": x_dev, **common})
    return in_maps


def kernel(x, target, embed_w, conv_w, conv_b, U_w, fc_w, fc_b, _want=None):
    x = np.asarray(x)
    target = np.asarray(target)
    embed_w = np.asarray(embed_w, dtype=np.float32)
    conv_w = np.asarray(conv_w, dtype=np.float32)
    conv_b = np.asarray(conv_b, dtype=np.float32)
    U_w = np.asarray(U_w, dtype=np.float32)
    fc_w = np.asarray(fc_w, dtype=np.float32)
    fc_b = np.asarray(fc_b, dtype=np.float32)

    nc, dt_mm = _get_nc()
    in_maps = _prep(dt_mm, x, embed_w, conv_w, conv_b, U_w, fc_w, fc_b)
    kw = _want if _want is not None else {}
    res = run_bass_kernel_spmd(nc, in_maps, list(range(NCORES)), **kw)

    logit = np.empty((B, Y), np.float32)
    for c in range(NCORES):
        lc = res.results[c]["logit"].reshape(SPC, YP)
        logit[c * SPC : (c + 1) * SPC] = lc[:, :Y]

    # cross-entropy on host (fp64 accumulate, tiny)
    lg = logit.astype(np.float64)
    m = lg.max(axis=1, keepdims=True)
    lse = m[:, 0] + np.log(np.exp(lg - m).sum(axis=1))
    picked = lg[np.arange(B), target.astype(np.int64)]
    loss = np.float32(-(picked - lse).mean())

    if _want is not None:
        return (logit, loss), res
    return (logit, loss)


# revision 17
# speedup vs baseline: 1.1039x; 1.1039x over previous
"""CAML (conv + label-wise attention) kernel for Trainium2, 8 NeuronCores.

Strategy: data-parallel over batch (16 samples -> 2 per core). Everything is
fused on-chip per sample:
  gather(embed) -> PE-transpose -> conv1d(K=3, as 3 accumulated matmuls)
  -> relu -> scores = U @ h (as (L,Y)-layout matmuls) -> exp (no max-sub;
  scores are tiny by construction) -> m/denominator via one matmul with a
  ones-column -> per-label dot with fc rows -> logits. Softmax normalization
  is folded into the final scale (logit = (fc . mU) / denom + b).
Loss (scalar cross-entropy over 16x1000 logits) is computed on host.

Emission is interleaved in l-blocks so the embedding gather (64 serial
SWDGE indirect DMAs per core, ~1.1us each on GpSimd) streams underneath
conv/scores compute instead of serializing the kernel front.
"""

import os
import numpy as np

import concourse.bass as bass
import concourse.bacc as bacc
from concourse import mybir
from concourse.bass_utils import run_bass_kernel_spmd
from concourse.masks import make_identity
from concourse.tile import TileContext

V, E, F, K, Y = 50000, 128, 256, 3, 1000
B, L = 16, 4096
YP = 1024          # labels padded to 8*128
NCORES = 8
SPC = B // NCORES  # samples per core
NLT = L // 128     # 32 l-tiles per sample
NLB = L // 1024    # 4 l-blocks per sample

F32 = mybir.dt.float32

_CACHE = {}


def _build(dt_mm):
    debug = os.environ.get("CAML_DEBUG", "0") == "1"
    nc = bacc.Bacc()

    x_d = nc.declare_dram_parameter("x", [SPC, 128, NLT], mybir.dt.int32, isOutput=False)
    emb_d = nc.declare_dram_parameter("embed", [V, E], F32, isOutput=False)
    wk_d = nc.declare_dram_parameter("convw", [K, 128, F], dt_mm, isOutput=False)
    cb_d = nc.declare_dram_parameter("convb", [128, 2], F32, isOutput=False)
    u_d = nc.declare_dram_parameter("uT", [2, 128, YP], dt_mm, isOutput=False)
    fcw_d = nc.declare_dram_parameter("fcw", [8, 128, F], F32, isOutput=False)
    fcb_d = nc.declare_dram_parameter("fcb", [128, 8], F32, isOutput=False)
    out_d = nc.declare_dram_parameter("logit", [SPC, 8, 128], F32, isOutput=True)
    if debug:
        dbg_h0 = nc.declare_dram_parameter("dbg_h0", [128, L + 2], F32, isOutput=True)
        dbg_hfl = nc.declare_dram_parameter("dbg_hfl", [128, 2, L], F32, isOutput=True)
        dbg_exps = nc.declare_dram_parameter("dbg_exps", [128, NLT, YP], F32, isOutput=True)
        dbg_pm = nc.declare_dram_parameter("dbg_pm", [128, F + 1], F32, isOutput=True)
        dbg_ht = nc.declare_dram_parameter("dbg_ht", [128, NLT, F + 1], F32, isOutput=True)

    with TileContext(nc) as tc:
        with (
            tc.tile_pool(name="singles", bufs=1) as singles,
            tc.tile_pool(name="p_emb", bufs=10) as p_emb,
            tc.tile_pool(name="p_idx", bufs=2) as p_idx,
            tc.tile_pool(name="p_epi", bufs=4) as p_epi,
            tc.tile_pool(name="p_out", bufs=1) as p_out,
            tc.tile_pool(name="ps_big", bufs=2, space="PSUM") as ps_big,
            tc.tile_pool(name="ps_t", bufs=2, space="PSUM") as ps_t,
            tc.tile_pool(name="ps_m", bufs=2, space="PSUM") as ps_m,
        ):
            # ---- constants / weights (loaded once) ----
            ident32 = singles.tile([128, 128], F32)
            make_identity(nc, ident32[:])
            if dt_mm != F32:
                ident_dt = singles.tile([128, 128], dt_mm)
                make_identity(nc, ident_dt[:])
            else:
                ident_dt = ident32

            wk_sb = singles.tile([128, K, F], dt_mm)
            for k in range(K):
                nc.sync.dma_start(out=wk_sb[:, k, :], in_=wk_d[k])
            cb_sb = singles.tile([128, 2], F32)
            nc.sync.dma_start(out=cb_sb[:], in_=cb_d[:])
            u_sb = singles.tile([128, 2, YP], dt_mm)
            for c in range(2):
                nc.sync.dma_start(out=u_sb[:, c, :], in_=u_d[c])
            fcw_sb = singles.tile([128, 8, F], F32)
            for t in range(8):
                nc.sync.dma_start(out=fcw_sb[:, t, :], in_=fcw_d[t])
            fcb_sb = singles.tile([128, 8], F32)
            nc.sync.dma_start(out=fcb_sb[:], in_=fcb_d[:])

            # ---- persistent per-sample workspaces ----
            h0 = singles.tile([128, L + 2], dt_mm)        # (E, Lpadded)
            nc.vector.memset(h0[:, 0:1], 0.0)
            nc.vector.memset(h0[:, L + 1 : L + 2], 0.0)
            h_fl = singles.tile([128, 2, L], dt_mm)       # (F-part, fc, L)
            ht = singles.tile([128, NLT, F + 1], dt_mm)   # (L-part, lt, F | ones)
            nc.vector.memset(ht[:, :, F : F + 1], 1.0)
            exps = singles.tile([128, NLT, YP], dt_mm)    # (L-part, lt, y)
            logit_sb = singles.tile([128, SPC, 8], F32)

            # idx for both samples upfront
            idxs = []
            for s in range(SPC):
                idx = p_idx.tile([128, NLT], mybir.dt.int32, tag="idx")
                nc.sync.dma_start(out=idx[:], in_=x_d[s])
                idxs.append(idx)

            nsamp = 1 if debug else SPC

            def gather_transpose(idx, j):
                """gather 128 tokens (chunk j) and transpose into h0."""
                embc = p_emb.tile([128, 128], F32, tag="embc")
                nc.gpsimd.indirect_dma_start(
                    out=embc[:], out_offset=None, in_=emb_d[:],
                    in_offset=bass.IndirectOffsetOnAxis(ap=idx[:, j : j + 1], axis=0),
                )
                pt = ps_t.tile([128, 128], F32, tag="pt")
                nc.tensor.transpose(out=pt[:], in_=embc[:], identity=ident32[:])
                col = 1 + j * 128
                nc.vector.tensor_copy(out=h0[:, col : col + 128], in_=pt[:])

            for s in range(nsamp):
                idx = idxs[s]
                # prefetch first block + one tile of lookahead
                for j in range(5):
                    gather_transpose(idx, j)

                for lb in range(NLB):
                    # prefetch gathers for next block (conv lb needs up to
                    # chunk 8*lb+8; scores need only lb's own chunks)
                    lo, hi = 8 * lb + 5, min(8 * lb + 13, NLT)
                    for j in range(lo, hi):
                        gather_transpose(idx, j)

                    # conv + relu for this 1024-wide l-block
                    for fc in range(2):
                        pc = ps_big.tile([128, 1024], F32, tag="big")
                        for half in range(2):
                            l0 = lb * 1024 + half * 512
                            for k in range(K):
                                nc.tensor.matmul(
                                    out=pc[:, half * 512 : half * 512 + 512],
                                    lhsT=wk_sb[:, k, fc * 128 : fc * 128 + 128],
                                    rhs=h0[:, l0 + k : l0 + k + 512],
                                    start=(k == 0),
                                    stop=(k == K - 1),
                                )
                        nc.scalar.activation(
                            out=h_fl[:, fc, lb * 1024 : (lb + 1) * 1024],
                            in_=pc[:],
                            func=mybir.ActivationFunctionType.Relu,
                            bias=cb_sb[:, fc : fc + 1],
                        )

                    # hT + scores + exp for the 8 l-tiles of this block
                    for lt in range(8 * lb, 8 * lb + 8):
                        for fc in range(2):
                            pt2 = ps_t.tile([128, 128], dt_mm, tag="pt")
                            nc.tensor.transpose(
                                out=pt2[:],
                                in_=h_fl[:, fc, lt * 128 : (lt + 1) * 128],
                                identity=ident_dt[:],
                            )
                            nc.vector.tensor_copy(
                                out=ht[:, lt, fc * 128 : fc * 128 + 128], in_=pt2[:]
                            )
                        psc = ps_big.tile([128, 1024], F32, tag="big")
                        for fc in range(2):
                            for yc in range(2):
                                nc.tensor.matmul(
                                    out=psc[:, yc * 512 : yc * 512 + 512],
                                    lhsT=h_fl[:, fc, lt * 128 : (lt + 1) * 128],
                                    rhs=u_sb[:, fc, yc * 512 : (yc + 1) * 512],
                                    start=(fc == 0),
                                    stop=(fc == 1),
                                )
                        nc.scalar.activation(
                            out=exps[:, lt, :], in_=psc[:],
                            func=mybir.ActivationFunctionType.Exp,
                        )

                if debug and s == 0:
                    nc.gpsimd.dma_start(out=dbg_h0[:], in_=h0[:])
                    nc.gpsimd.dma_start(out=dbg_hfl[:], in_=h_fl[:])
                    nc.gpsimd.dma_start(out=dbg_ht[:], in_=ht[:])
                    nc.gpsimd.dma_start(out=dbg_exps[:], in_=exps[:])

                # ---- m matmuls + per-label epilogue ----
                for yt in range(8):
                    pm = ps_m.tile([128, F + 1], F32, tag="pm")
                    for lt in range(NLT):
                        nc.tensor.matmul(
                            out=pm[:],
                            lhsT=exps[:, lt, yt * 128 : (yt + 1) * 128],
                            rhs=ht[:, lt, :],
                            start=(lt == 0),
                            stop=(lt == NLT - 1),
                        )
                    if debug and s == 0 and yt == 0:
                        pmst = p_epi.tile([128, F + 1], F32, tag="pmdump")
                        nc.vector.tensor_copy(out=pmst[:], in_=pm[:])
                        nc.gpsimd.dma_start(out=dbg_pm[:], in_=pmst[:])
                    # logit[y] = (fc_w[y] . mU[y]) / denom[y] + fc_b[y]
                    prod = p_epi.tile([128, F], F32, tag="prod")
                    nc.vector.tensor_mul(prod[:], pm[:, 0:F], fcw_sb[:, yt, :])
                    ssum = p_epi.tile([128, 1], F32, tag="ssum")
                    nc.vector.reduce_sum(
                        out=ssum[:], in_=prod[:], axis=mybir.AxisListType.X
                    )
                    rcp = p_epi.tile([128, 1], F32, tag="rcp")
                    nc.vector.reciprocal(out=rcp[:], in_=pm[:, F : F + 1])
                    tl = p_epi.tile([128, 1], F32, tag="tl")
                    nc.vector.tensor_mul(tl[:], ssum[:], rcp[:])
                    nc.vector.tensor_add(
                        logit_sb[:, s, yt : yt + 1], tl[:], fcb_sb[:, yt : yt + 1]
                    )

            # ---- transpose logits to (s*t, p) rows and store ----
            po = ps_m.tile([SPC * 8, 128], F32, tag="pm")
            nc.tensor.transpose(
                out=po[:],
                in_=logit_sb[:].rearrange("p s t -> p (s t)"),
                identity=ident32[:],
            )
            lout = p_out.tile([SPC * 8, 128], F32)
            nc.vector.tensor_copy(out=lout[:], in_=po[:])
            nc.sync.dma_start(
                out=out_d[:].rearrange("s t p -> (s t) p"), in_=lout[:]
            )

    nc.compile()
    return nc


def _get_nc():
    dt_name = os.environ.get("CAML_DT", "bf16")
    if dt_name not in _CACHE:
        dt_mm = {"bf16": mybir.dt.bfloat16, "f32": F32,
                 "f32r": mybir.dt.float32r}[dt_name]
        _CACHE[dt_name] = (_build(dt_mm), dt_mm)
    return _CACHE[dt_name]


def _prep(dt_mm, x, embed_w, conv_w, conv_b, U_w, fc_w, fc_b):
    npdt = np.float32 if dt_mm in (F32, mybir.dt.float32r) else mybir.dt.np(dt_mm)
    wkT = np.ascontiguousarray(conv_w.transpose(2, 1, 0)).astype(npdt)  # (K,E,F)
    cb = np.ascontiguousarray(conv_b.reshape(2, 128).T).astype(np.float32)
    u_pad = np.zeros((YP, F), np.float32)
    u_pad[:Y] = U_w
    uT = np.ascontiguousarray(u_pad.T.reshape(2, 128, YP)).astype(npdt)
    fc_pad = np.zeros((YP, F), np.float32)
    fc_pad[:Y] = fc_w
    fcw = np.ascontiguousarray(fc_pad.reshape(8, 128, F))
    fcb_pad = np.zeros((YP,), np.float32)
    fcb_pad[:Y] = fc_b
    fcb = np.ascontiguousarray(fcb_pad.reshape(8, 128).T)
    common = {
        "embed": np.ascontiguousarray(embed_w, dtype=np.float32),
        "convw": wkT, "convb": cb, "uT": uT, "fcw": fcw, "fcb": fcb,
    }
    in_maps = []
    for c in range(NCORES):
        xs = x[c * SPC : (c + 1) * SPC].astype(np.int32)
        x_dev = np.ascontiguousarray(xs.reshape(SPC, NLT, 128).transpose(0, 2, 1))
        in_maps.append({"x": x_dev, **common})
    return in_maps


def kernel(x, target, embed_w, conv_w, conv_b, U_w, fc_w, fc_b, _want=None):
    x = np.asarray(x)
    target = np.asarray(target)
    embed_w = np.asarray(embed_w, dtype=np.float32)
    conv_w = np.asarray(conv_w, dtype=np.float32)
    conv_b = np.asarray(conv_b, dtype=np.float32)
    U_w = np.asarray(U_w, dtype=np.float32)
    fc_w = np.asarray(fc_w, dtype=np.float32)
    fc_b = np.asarray(fc_b, dtype=np.float32)

    nc, dt_mm = _get_nc()
    in_maps = _prep(dt_mm, x, embed_w, conv_w, conv_b, U_w, fc_w, fc_b)
    kw = _want if _want is not None else {}
    res = run_bass_kernel_spmd(nc, in_maps, list(range(NCORES)), **kw)

    logit = np.empty((B, Y), np.float32)
    for c in range(NCORES):
        lc = res.results[c]["logit"].reshape(SPC, YP)
        logit[c * SPC : (c + 1) * SPC] = lc[:, :Y]

    # cross-entropy on host (fp64 accumulate, tiny)
    lg = logit.astype(np.float64)
    m = lg.max(axis=1, keepdims=True)
    lse = m[:, 0] + np.log(np.exp(lg - m).sum(axis=1))
    picked = lg[np.arange(B), target.astype(np.int64)]
    loss = np.float32(-(picked - lse).mean())

    if _want is not None:
        return (logit, loss), res
    return (logit, loss)


# revision 21
# speedup vs baseline: 1.1470x; 1.0391x over previous
"""CAML (conv + label-wise attention) kernel for Trainium2, 8 NeuronCores.

Strategy: data-parallel over batch (16 samples -> 2 per core). Everything is
fused on-chip per sample:
  gather(embed) -> PE-transpose -> conv1d(K=3, as 3 accumulated matmuls)
  -> relu -> scores = U @ h (as (L,Y)-layout matmuls) -> exp (no max-sub;
  scores are tiny by construction) -> m/denominator via one matmul with a
  ones-column -> per-label dot with fc rows -> logits. Softmax normalization
  is folded into the final scale (logit = (fc . mU) / denom + b).
Loss (scalar cross-entropy over 16x1000 logits) is computed on host.

Emission is interleaved in l-blocks so the embedding gather (64 serial
SWDGE indirect DMAs per core, ~1.1us each on GpSimd) streams underneath
conv/scores compute instead of serializing the kernel front.
"""

import os
import numpy as np

import concourse.bass as bass
import concourse.bacc as bacc
from concourse import mybir
from concourse.bass_utils import run_bass_kernel_spmd
from concourse.masks import make_identity
from concourse.tile import TileContext

V, E, F, K, Y = 50000, 128, 256, 3, 1000
B, L = 16, 4096
YP = 1024          # labels padded to 8*128
NCORES = 8
SPC = B // NCORES  # samples per core
NLT = L // 128     # 32 l-tiles per sample
NLB = L // 1024    # 4 l-blocks per sample

F32 = mybir.dt.float32

_CACHE = {}


def _build(dt_mm):
    debug = os.environ.get("CAML_DEBUG", "0") == "1"
    nc = bacc.Bacc()

    x_d = nc.declare_dram_parameter("x", [SPC, 128, NLT], mybir.dt.int32, isOutput=False)
    emb_d = nc.declare_dram_parameter("embed", [V, E], F32, isOutput=False)
    wk_d = nc.declare_dram_parameter("convw", [K, 128, F], dt_mm, isOutput=False)
    cb_d = nc.declare_dram_parameter("convb", [128, 2], F32, isOutput=False)
    u_d = nc.declare_dram_parameter("uT", [2, 128, YP], dt_mm, isOutput=False)
    fcw_d = nc.declare_dram_parameter("fcw", [8, 128, F], F32, isOutput=False)
    fcb_d = nc.declare_dram_parameter("fcb", [128, 8], F32, isOutput=False)
    out_d = nc.declare_dram_parameter("logit", [SPC, 8, 128], F32, isOutput=True)
    if debug:
        dbg_h0 = nc.declare_dram_parameter("dbg_h0", [128, L + 2], F32, isOutput=True)
        dbg_hfl = nc.declare_dram_parameter("dbg_hfl", [128, 2, L], F32, isOutput=True)
        dbg_exps = nc.declare_dram_parameter("dbg_exps", [128, NLT, YP], F32, isOutput=True)
        dbg_pm = nc.declare_dram_parameter("dbg_pm", [128, F + 1], F32, isOutput=True)
        dbg_ht = nc.declare_dram_parameter("dbg_ht", [128, NLT, F + 1], F32, isOutput=True)

    with TileContext(nc) as tc:
        with (
            tc.tile_pool(name="singles", bufs=1) as singles,
            tc.tile_pool(name="p_emb", bufs=40) as p_emb,
            tc.tile_pool(name="p_idx", bufs=2) as p_idx,
            tc.tile_pool(name="p_epi", bufs=4) as p_epi,
            tc.tile_pool(name="p_out", bufs=1) as p_out,
            tc.tile_pool(name="ps_big", bufs=2, space="PSUM") as ps_big,
            tc.tile_pool(name="ps_t", bufs=2, space="PSUM") as ps_t,
            tc.tile_pool(name="ps_m", bufs=2, space="PSUM") as ps_m,
        ):
            # ---- idx loads first: they gate the gathers, and the HWDGE
            # queue is FIFO — behind the 1.6MB of weight loads they'd stall
            # the kernel front by ~15us ----
            idxs = []
            for s in range(SPC):
                idx = p_idx.tile([128, NLT], mybir.dt.int32, tag="idx")
                nc.sync.dma_start(out=idx[:], in_=x_d[s])
                idxs.append(idx)

            # ---- constants / weights (loaded once) ----
            ident32 = singles.tile([128, 128], F32)
            make_identity(nc, ident32[:])
            if dt_mm != F32:
                ident_dt = singles.tile([128, 128], dt_mm)
                make_identity(nc, ident_dt[:])
            else:
                ident_dt = ident32

            wk_sb = singles.tile([128, K, F], dt_mm)
            for k in range(K):
                nc.sync.dma_start(out=wk_sb[:, k, :], in_=wk_d[k])
            cb_sb = singles.tile([128, 2], F32)
            nc.sync.dma_start(out=cb_sb[:], in_=cb_d[:])
            u_sb = singles.tile([128, 2, YP], dt_mm)
            for c in range(2):
                nc.sync.dma_start(out=u_sb[:, c, :], in_=u_d[c])
            fcw_sb = singles.tile([128, 8, F], F32)
            for t in range(8):
                nc.sync.dma_start(out=fcw_sb[:, t, :], in_=fcw_d[t])
            fcb_sb = singles.tile([128, 8], F32)
            nc.sync.dma_start(out=fcb_sb[:], in_=fcb_d[:])

            # ---- persistent per-sample workspaces ----
            h0 = singles.tile([128, L + 2], dt_mm)        # (E, Lpadded)
            nc.vector.memset(h0[:, 0:1], 0.0)
            nc.vector.memset(h0[:, L + 1 : L + 2], 0.0)
            h_fl = singles.tile([128, 2, L], dt_mm)       # (F-part, fc, L)
            ht = singles.tile([128, NLT, F + 1], dt_mm)   # (L-part, lt, F | ones)
            nc.vector.memset(ht[:, :, F : F + 1], 1.0)
            exps = singles.tile([128, NLT, YP], dt_mm)    # (L-part, lt, y)
            logit_sb = singles.tile([128, SPC, 8], F32)

            nsamp = 1 if debug else SPC

            def gather_transpose(idx, j):
                """gather 128 tokens (chunk j) and transpose into h0."""
                embc = p_emb.tile([128, 128], F32, tag="embc")
                nc.gpsimd.indirect_dma_start(
                    out=embc[:], out_offset=None, in_=emb_d[:],
                    in_offset=bass.IndirectOffsetOnAxis(ap=idx[:, j : j + 1], axis=0),
                )
                pt = ps_t.tile([128, 128], F32, tag="pt")
                nc.tensor.transpose(out=pt[:], in_=embc[:], identity=ident32[:])
                col = 1 + j * 128
                nc.vector.tensor_copy(out=h0[:, col : col + 128], in_=pt[:])

            for s in range(nsamp):
                idx = idxs[s]
                # prefetch first block + one tile of lookahead
                for j in range(5):
                    gather_transpose(idx, j)

                for lb in range(NLB):
                    # prefetch gathers for next block (conv lb needs up to
                    # chunk 8*lb+8; scores need only lb's own chunks)
                    lo, hi = 8 * lb + 5, min(8 * lb + 13, NLT)
                    for j in range(lo, hi):
                        gather_transpose(idx, j)

                    # conv + relu for this 1024-wide l-block
                    for fc in range(2):
                        pc = ps_big.tile([128, 1024], F32, tag="big")
                        for half in range(2):
                            l0 = lb * 1024 + half * 512
                            for k in range(K):
                                nc.tensor.matmul(
                                    out=pc[:, half * 512 : half * 512 + 512],
                                    lhsT=wk_sb[:, k, fc * 128 : fc * 128 + 128],
                                    rhs=h0[:, l0 + k : l0 + k + 512],
                                    start=(k == 0),
                                    stop=(k == K - 1),
                                )
                        # relu+bias on DVE (keeps ACT free for the exps)
                        nc.vector.tensor_scalar(
                            out=h_fl[:, fc, lb * 1024 : (lb + 1) * 1024],
                            in0=pc[:],
                            scalar1=cb_sb[:, fc : fc + 1],
                            scalar2=0.0,
                            op0=mybir.AluOpType.add,
                            op1=mybir.AluOpType.max,
                        )

                    # hT + scores + exp for the 8 l-tiles of this block
                    for lt in range(8 * lb, 8 * lb + 8):
                        for fc in range(2):
                            pt2 = ps_t.tile([128, 128], dt_mm, tag="pt")
                            nc.tensor.transpose(
                                out=pt2[:],
                                in_=h_fl[:, fc, lt * 128 : (lt + 1) * 128],
                                identity=ident_dt[:],
                            )
                            nc.vector.tensor_copy(
                                out=ht[:, lt, fc * 128 : fc * 128 + 128], in_=pt2[:]
                            )
                        psc = ps_big.tile([128, 1024], F32, tag="big")
                        for fc in range(2):
                            for yc in range(2):
                                nc.tensor.matmul(
                                    out=psc[:, yc * 512 : yc * 512 + 512],
                                    lhsT=h_fl[:, fc, lt * 128 : (lt + 1) * 128],
                                    rhs=u_sb[:, fc, yc * 512 : (yc + 1) * 512],
                                    start=(fc == 0),
                                    stop=(fc == 1),
                                )
                        nc.scalar.activation(
                            out=exps[:, lt, :], in_=psc[:],
                            func=mybir.ActivationFunctionType.Exp,
                        )

                if debug and s == 0:
                    nc.gpsimd.dma_start(out=dbg_h0[:], in_=h0[:])
                    nc.gpsimd.dma_start(out=dbg_hfl[:], in_=h_fl[:])
                    nc.gpsimd.dma_start(out=dbg_ht[:], in_=ht[:])
                    nc.gpsimd.dma_start(out=dbg_exps[:], in_=exps[:])

                # ---- m matmuls + per-label epilogue ----
                for yt in range(8):
                    pm = ps_m.tile([128, F + 1], F32, tag="pm")
                    for lt in range(NLT):
                        nc.tensor.matmul(
                            out=pm[:],
                            lhsT=exps[:, lt, yt * 128 : (yt + 1) * 128],
                            rhs=ht[:, lt, :],
                            start=(lt == 0),
                            stop=(lt == NLT - 1),
                        )
                    if debug and s == 0 and yt == 0:
                        pmst = p_epi.tile([128, F + 1], F32, tag="pmdump")
                        nc.vector.tensor_copy(out=pmst[:], in_=pm[:])
                        nc.gpsimd.dma_start(out=dbg_pm[:], in_=pmst[:])
                    # logit[y] = (fc_w[y] . mU[y]) / denom[y] + fc_b[y]
                    prod = p_epi.tile([128, F], F32, tag="prod")
                    nc.vector.tensor_mul(prod[:], pm[:, 0:F], fcw_sb[:, yt, :])
                    ssum = p_epi.tile([128, 1], F32, tag="ssum")
                    nc.vector.reduce_sum(
                        out=ssum[:], in_=prod[:], axis=mybir.AxisListType.X
                    )
                    rcp = p_epi.tile([128, 1], F32, tag="rcp")
                    nc.vector.reciprocal(out=rcp[:], in_=pm[:, F : F + 1])
                    tl = p_epi.tile([128, 1], F32, tag="tl")
                    nc.vector.tensor_mul(tl[:], ssum[:], rcp[:])
                    nc.vector.tensor_add(
                        logit_sb[:, s, yt : yt + 1], tl[:], fcb_sb[:, yt : yt + 1]
                    )

            # ---- transpose logits to (s*t, p) rows and store ----
            po = ps_m.tile([SPC * 8, 128], F32, tag="pm")
            nc.tensor.transpose(
                out=po[:],
                in_=logit_sb[:].rearrange("p s t -> p (s t)"),
                identity=ident32[:],
            )
            lout = p_out.tile([SPC * 8, 128], F32)
            nc.vector.tensor_copy(out=lout[:], in_=po[:])
            nc.sync.dma_start(
                out=out_d[:].rearrange("s t p -> (s t) p"), in_=lout[:]
            )

    nc.compile()
    return nc


def _get_nc():
    dt_name = os.environ.get("CAML_DT", "bf16")
    if dt_name not in _CACHE:
        dt_mm = {"bf16": mybir.dt.bfloat16, "f32": F32,
                 "f32r": mybir.dt.float32r}[dt_name]
        _CACHE[dt_name] = (_build(dt_mm), dt_mm)
    return _CACHE[dt_name]


def _prep(dt_mm, x, embed_w, conv_w, conv_b, U_w, fc_w, fc_b):
    npdt = np.float32 if dt_mm in (F32, mybir.dt.float32r) else mybir.dt.np(dt_mm)
    wkT = np.ascontiguousarray(conv_w.transpose(2, 1, 0)).astype(npdt)  # (K,E,F)
    cb = np.ascontiguousarray(conv_b.reshape(2, 128).T).astype(np.float32)
    u_pad = np.zeros((YP, F), np.float32)
    u_pad[:Y] = U_w
    uT = np.ascontiguousarray(u_pad.T.reshape(2, 128, YP)).astype(npdt)
    fc_pad = np.zeros((YP, F), np.float32)
    fc_pad[:Y] = fc_w
    fcw = np.ascontiguousarray(fc_pad.reshape(8, 128, F))
    fcb_pad = np.zeros((YP,), np.float32)
    fcb_pad[:Y] = fc_b
    fcb = np.ascontiguousarray(fcb_pad.reshape(8, 128).T)
    common = {
        "embed": np.ascontiguousarray(embed_w, dtype=np.float32),
        "convw": wkT, "convb": cb, "uT": uT, "fcw": fcw, "fcb": fcb,
    }
    in_maps = []
    for c in range(NCORES):
        xs = x[c * SPC : (c + 1) * SPC].astype(np.int32)
        x_dev = np.ascontiguousarray(xs.reshape(SPC, NLT, 128).transpose(0, 2, 1))
        in_maps.append({"x": x_dev, **common})
    return in_maps


def kernel(x, target, embed_w, conv_w, conv_b, U_w, fc_w, fc_b, _want=None):
    x = np.asarray(x)
    target = np.asarray(target)
    embed_w = np.asarray(embed_w, dtype=np.float32)
    conv_w = np.asarray(conv_w, dtype=np.float32)
    conv_b = np.asarray(conv_b, dtype=np.float32)
    U_w = np.asarray(U_w, dtype=np.float32)
    fc_w = np.asarray(fc_w, dtype=np.float32)
    fc_b = np.asarray(fc_b, dtype=np.float32)

    nc, dt_mm = _get_nc()
    in_maps = _prep(dt_mm, x, embed_w, conv_w, conv_b, U_w, fc_w, fc_b)
    kw = _want if _want is not None else {}
    res = run_bass_kernel_spmd(nc, in_maps, list(range(NCORES)), **kw)

    logit = np.empty((B, Y), np.float32)
    for c in range(NCORES):
        lc = res.results[c]["logit"].reshape(SPC, YP)
        logit[c * SPC : (c + 1) * SPC] = lc[:, :Y]

    # cross-entropy on host (fp64 accumulate, tiny)
    lg = logit.astype(np.float64)
    m = lg.max(axis=1, keepdims=True)
    lse = m[:, 0] + np.log(np.exp(lg - m).sum(axis=1))
    picked = lg[np.arange(B), target.astype(np.int64)]
    loss = np.float32(-(picked - lse).mean())

    if _want is not None:
        return (logit, loss), res
    return (logit, loss)


# revision 25
# speedup vs baseline: 1.1861x; 1.0341x over previous
"""CAML (conv + label-wise attention) kernel for Trainium2, 8 NeuronCores.

Strategy: data-parallel over batch (16 samples -> 2 per core). Everything is
fused on-chip per sample:
  gather(embed) -> PE-transpose -> conv1d(K=3, as 3 accumulated matmuls)
  -> relu -> scores = U @ h (as (L,Y)-layout matmuls) -> exp (no max-sub;
  scores are tiny by construction) -> m/denominator via one matmul with a
  ones-column -> per-label dot with fc rows -> logits. Softmax normalization
  is folded into the final scale (logit = (fc . mU) / denom + b).
Loss (scalar cross-entropy over 16x1000 logits) is computed on host.

Pipelining: the embedding gather is 64 serial SWDGE indirect DMAs (~1.1us of
GpSimd each) forming a ~70us backbone; compute is emitted so it hides under
it. The m-matmul contraction over L is split into two halves (accumulated via
SBUF) so the second sample's scores/exp (ACT) overlap the first sample's m
(PE), with the shared exps buffer recycled in lt-halves.
"""

import os
import numpy as np

import concourse.bass as bass
import concourse.bacc as bacc
from concourse import mybir
from concourse.bass_utils import run_bass_kernel_spmd
from concourse.masks import make_identity
from concourse.tile import TileContext

V, E, F, K, Y = 50000, 128, 256, 3, 1000
B, L = 16, 4096
YP = 1024          # labels padded to 8*128
NCORES = 8
SPC = B // NCORES  # samples per core
NLT = L // 128     # 32 l-tiles per sample
NLB = L // 1024    # 4 l-blocks per sample

F32 = mybir.dt.float32

_CACHE = {}


def _build(dt_mm):
    debug = os.environ.get("CAML_DEBUG", "0") == "1"
    nc = bacc.Bacc()

    x_d = nc.declare_dram_parameter("x", [SPC, 128, NLT], mybir.dt.int32, isOutput=False)
    emb_d = nc.declare_dram_parameter("embed", [V, E], F32, isOutput=False)
    wk_d = nc.declare_dram_parameter("convw", [K, 128, F], dt_mm, isOutput=False)
    cb_d = nc.declare_dram_parameter("convb", [128, 2], F32, isOutput=False)
    u_d = nc.declare_dram_parameter("uT", [2, 128, YP], dt_mm, isOutput=False)
    fcw_d = nc.declare_dram_parameter("fcw", [8, 128, F], F32, isOutput=False)
    fcb_d = nc.declare_dram_parameter("fcb", [128, 8], F32, isOutput=False)
    out_d = nc.declare_dram_parameter("logit", [SPC, 8, 128], F32, isOutput=True)
    if debug:
        dbg_h0 = nc.declare_dram_parameter("dbg_h0", [128, L + 2], F32, isOutput=True)
        dbg_hfl = nc.declare_dram_parameter("dbg_hfl", [128, 2, L], F32, isOutput=True)
        dbg_exps = nc.declare_dram_parameter("dbg_exps", [128, NLT, YP], F32, isOutput=True)
        dbg_ht = nc.declare_dram_parameter("dbg_ht", [128, NLT, F + 1], F32, isOutput=True)

    with TileContext(nc) as tc:
        with (
            tc.tile_pool(name="singles", bufs=1) as singles,
            tc.tile_pool(name="p_ht", bufs=2) as p_ht,
            tc.tile_pool(name="p_emb", bufs=24) as p_emb,
            tc.tile_pool(name="p_idx", bufs=2) as p_idx,
            tc.tile_pool(name="p_epi", bufs=4) as p_epi,
            tc.tile_pool(name="p_out", bufs=1) as p_out,
            tc.tile_pool(name="ps_big", bufs=2, space="PSUM") as ps_big,
            tc.tile_pool(name="ps_t", bufs=2, space="PSUM") as ps_t,
            tc.tile_pool(name="ps_m", bufs=2, space="PSUM") as ps_m,
        ):
            # idx loads first: they gate the gathers; behind the weight
            # loads on the FIFO HWDGE queue they'd stall the kernel front.
            idxs = []
            for s in range(SPC):
                idx = p_idx.tile([128, NLT], mybir.dt.int32, tag="idx")
                nc.sync.dma_start(out=idx[:], in_=x_d[s])
                idxs.append(idx)

            ident32 = singles.tile([128, 128], F32)
            make_identity(nc, ident32[:])
            if dt_mm != F32:
                ident_dt = singles.tile([128, 128], dt_mm)
                make_identity(nc, ident_dt[:])
            else:
                ident_dt = ident32

            wk_sb = singles.tile([128, K, F], dt_mm)
            for k in range(K):
                nc.sync.dma_start(out=wk_sb[:, k, :], in_=wk_d[k])
            cb_sb = singles.tile([128, 2], F32)
            nc.sync.dma_start(out=cb_sb[:], in_=cb_d[:])
            u_sb = singles.tile([128, 2, YP], dt_mm)
            for c in range(2):
                nc.sync.dma_start(out=u_sb[:, c, :], in_=u_d[c])
            fcw_sb = singles.tile([128, 8, F], F32)
            for t in range(8):
                nc.sync.dma_start(out=fcw_sb[:, t, :], in_=fcw_d[t])
            fcb_sb = singles.tile([128, 8], F32)
            nc.sync.dma_start(out=fcb_sb[:], in_=fcb_d[:])

            # shared workspaces: h0/h_fl/exps single (recycled in-order),
            # ht double (next sample's hT overlaps this sample's m)
            h0 = singles.tile([128, L + 2], dt_mm)
            nc.vector.memset(h0[:, 0:1], 0.0)
            nc.vector.memset(h0[:, L + 1 : L + 2], 0.0)
            h_fl = singles.tile([128, 2, L], dt_mm)
            exps = singles.tile([128, NLT, YP], dt_mm)
            mu_sb = singles.tile([128, 8, F + 1], F32)
            logit_sb = singles.tile([128, SPC, 8], F32)

            def gather_transpose(idx, j):
                embc = p_emb.tile([128, 128], F32, tag="embc")
                nc.gpsimd.indirect_dma_start(
                    out=embc[:], out_offset=None, in_=emb_d[:],
                    in_offset=bass.IndirectOffsetOnAxis(ap=idx[:, j : j + 1], axis=0),
                )
                pt = ps_t.tile([128, 128], F32, tag="pt")
                nc.tensor.transpose(out=pt[:], in_=embc[:], identity=ident32[:])
                col = 1 + j * 128
                nc.vector.tensor_copy(out=h0[:, col : col + 128], in_=pt[:])

            def new_ht():
                ht = p_ht.tile([128, NLT, F + 1], dt_mm, tag="ht")
                nc.vector.memset(ht[:, :, F : F + 1], 1.0)
                return ht

            gptr = {0: 0, 1: 0}

            def block(s, lb, ht, pre=4):
                """gathers (with `pre` tiles of lookahead), conv+relu, hT
                for l-block lb of sample s."""
                idx = idxs[s]
                tgt = min(8 * lb + 9 + pre, NLT)
                for j in range(gptr[s], tgt):
                    gather_transpose(idx, j)
                gptr[s] = tgt
                for fc in range(2):
                    pc = ps_big.tile([128, 1024], F32, tag="big")
                    for half in range(2):
                        l0 = lb * 1024 + half * 512
                        for k in range(K):
                            nc.tensor.matmul(
                                out=pc[:, half * 512 : half * 512 + 512],
                                lhsT=wk_sb[:, k, fc * 128 : fc * 128 + 128],
                                rhs=h0[:, l0 + k : l0 + k + 512],
                                start=(k == 0),
                                stop=(k == K - 1),
                            )
                    nc.vector.tensor_scalar(
                        out=h_fl[:, fc, lb * 1024 : (lb + 1) * 1024],
                        in0=pc[:],
                        scalar1=cb_sb[:, fc : fc + 1],
                        scalar2=0.0,
                        op0=mybir.AluOpType.add,
                        op1=mybir.AluOpType.max,
                    )
                for lt in range(8 * lb, 8 * lb + 8):
                    for fc in range(2):
                        pt2 = ps_t.tile([128, 128], dt_mm, tag="pt")
                        nc.tensor.transpose(
                            out=pt2[:],
                            in_=h_fl[:, fc, lt * 128 : (lt + 1) * 128],
                            identity=ident_dt[:],
                        )
                        nc.vector.tensor_copy(
                            out=ht[:, lt, fc * 128 : fc * 128 + 128], in_=pt2[:]
                        )

            def scores(lts):
                for lt in lts:
                    psc = ps_big.tile([128, 1024], F32, tag="big")
                    for fc in range(2):
                        for yc in range(2):
                            nc.tensor.matmul(
                                out=psc[:, yc * 512 : yc * 512 + 512],
                                lhsT=h_fl[:, fc, lt * 128 : (lt + 1) * 128],
                                rhs=u_sb[:, fc, yc * 512 : (yc + 1) * 512],
                                start=(fc == 0),
                                stop=(fc == 1),
                            )
                    nc.scalar.activation(
                        out=exps[:, lt, :], in_=psc[:],
                        func=mybir.ActivationFunctionType.Exp,
                    )

            def m_pass1(ht):
                """first L-half of the m contraction -> mu_sb"""
                for yt in range(8):
                    pm = ps_m.tile([128, F + 1], F32, tag="pm")
                    for lt in range(NLT // 2):
                        nc.tensor.matmul(
                            out=pm[:],
                            lhsT=exps[:, lt, yt * 128 : (yt + 1) * 128],
                            rhs=ht[:, lt, :],
                            start=(lt == 0),
                            stop=(lt == NLT // 2 - 1),
                        )
                    nc.vector.tensor_copy(out=mu_sb[:, yt, :], in_=pm[:])

            def m_pass2(s, ht):
                """second L-half + epilogue -> logits"""
                for yt in range(8):
                    pm = ps_m.tile([128, F + 1], F32, tag="pm")
                    for lt in range(NLT // 2, NLT):
                        nc.tensor.matmul(
                            out=pm[:],
                            lhsT=exps[:, lt, yt * 128 : (yt + 1) * 128],
                            rhs=ht[:, lt, :],
                            start=(lt == NLT // 2),
                            stop=(lt == NLT - 1),
                        )
                    tot = p_epi.tile([128, F + 1], F32, tag="tot")
                    nc.vector.tensor_add(tot[:], mu_sb[:, yt, :], pm[:])
                    prod = p_epi.tile([128, F], F32, tag="prod")
                    nc.vector.tensor_mul(prod[:], tot[:, 0:F], fcw_sb[:, yt, :])
                    ssum = p_epi.tile([128, 1], F32, tag="ssum")
                    nc.vector.reduce_sum(
                        out=ssum[:], in_=prod[:], axis=mybir.AxisListType.X
                    )
                    rcp = p_epi.tile([128, 1], F32, tag="rcp")
                    nc.vector.reciprocal(out=rcp[:], in_=tot[:, F : F + 1])
                    tl = p_epi.tile([128, 1], F32, tag="tl")
                    nc.vector.tensor_mul(tl[:], ssum[:], rcp[:])
                    nc.vector.tensor_add(
                        logit_sb[:, s, yt : yt + 1], tl[:], fcb_sb[:, yt : yt + 1]
                    )

            # ---- pipelined emission ----
            ht0 = new_ht()
            for lb in range(NLB):
                block(0, lb, ht0, pre=4)
                scores(range(8 * lb, 8 * lb + 8))

            if debug:
                nc.gpsimd.dma_start(out=dbg_h0[:], in_=h0[:])
                nc.gpsimd.dma_start(out=dbg_hfl[:], in_=h_fl[:])
                nc.gpsimd.dma_start(out=dbg_ht[:], in_=ht0[:])
                nc.gpsimd.dma_start(out=dbg_exps[:], in_=exps[:])
                m_pass1(ht0)
                m_pass2(0, ht0)
            else:
                ht1 = new_ht()
                block(1, 0, ht1, pre=4)
                block(1, 1, ht1, pre=4)
                m_pass1(ht0)               # frees exps lt0-15 (s0)
                scores(range(0, 8))        # s1 scores start refilling
                scores(range(8, 16))
                block(1, 2, ht1, pre=4)
                m_pass2(0, ht0)            # frees exps lt16-31 (s0)
                block(1, 3, ht1)
                scores(range(16, 24))
                scores(range(24, 32))
                m_pass1(ht1)
                m_pass2(1, ht1)

            # ---- transpose logits to (s*t, p) rows and store ----
            po = ps_m.tile([SPC * 8, 128], F32, tag="pm")
            nc.tensor.transpose(
                out=po[:],
                in_=logit_sb[:].rearrange("p s t -> p (s t)"),
                identity=ident32[:],
            )
            lout = p_out.tile([SPC * 8, 128], F32)
            nc.vector.tensor_copy(out=lout[:], in_=po[:])
            nc.sync.dma_start(
                out=out_d[:].rearrange("s t p -> (s t) p"), in_=lout[:]
            )

    nc.compile()
    return nc


def _get_nc():
    dt_name = os.environ.get("CAML_DT", "bf16")
    if dt_name not in _CACHE:
        dt_mm = {"bf16": mybir.dt.bfloat16, "f32": F32,
                 "f32r": mybir.dt.float32r}[dt_name]
        _CACHE[dt_name] = (_build(dt_mm), dt_mm)
    return _CACHE[dt_name]


def _prep(dt_mm, x, embed_w, conv_w, conv_b, U_w, fc_w, fc_b):
    npdt = np.float32 if dt_mm in (F32, mybir.dt.float32r) else mybir.dt.np(dt_mm)
    wkT = np.ascontiguousarray(conv_w.transpose(2, 1, 0)).astype(npdt)  # (K,E,F)
    cb = np.ascontiguousarray(conv_b.reshape(2, 128).T).astype(np.float32)
    u_pad = np.zeros((YP, F), np.float32)
    u_pad[:Y] = U_w
    uT = np.ascontiguousarray(u_pad.T.reshape(2, 128, YP)).astype(npdt)
    fc_pad = np.zeros((YP, F), np.float32)
    fc_pad[:Y] = fc_w
    fcw = np.ascontiguousarray(fc_pad.reshape(8, 128, F))
    fcb_pad = np.zeros((YP,), np.float32)
    fcb_pad[:Y] = fc_b
    fcb = np.ascontiguousarray(fcb_pad.reshape(8, 128).T)
    common = {
        "embed": np.ascontiguousarray(embed_w, dtype=np.float32),
        "convw": wkT, "convb": cb, "uT": uT, "fcw": fcw, "fcb": fcb,
    }
    in_maps = []
    for c in range(NCORES):
        xs = x[c * SPC : (c + 1) * SPC].astype(np.int32)
        x_dev = np.ascontiguousarray(xs.reshape(SPC, NLT, 128).transpose(0, 2, 1))
        in_maps.append({"x": x_dev, **common})
    return in_maps


def kernel(x, target, embed_w, conv_w, conv_b, U_w, fc_w, fc_b, _want=None):
    x = np.asarray(x)
    target = np.asarray(target)
    embed_w = np.asarray(embed_w, dtype=np.float32)
    conv_w = np.asarray(conv_w, dtype=np.float32)
    conv_b = np.asarray(conv_b, dtype=np.float32)
    U_w = np.asarray(U_w, dtype=np.float32)
    fc_w = np.asarray(fc_w, dtype=np.float32)
    fc_b = np.asarray(fc_b, dtype=np.float32)

    nc, dt_mm = _get_nc()
    in_maps = _prep(dt_mm, x, embed_w, conv_w, conv_b, U_w, fc_w, fc_b)
    kw = _want if _want is not None else {}
    res = run_bass_kernel_spmd(nc, in_maps, list(range(NCORES)), **kw)

    logit = np.empty((B, Y), np.float32)
    for c in range(NCORES):
        lc = res.results[c]["logit"].reshape(SPC, YP)
        logit[c * SPC : (c + 1) * SPC] = lc[:, :Y]

    # cross-entropy on host (fp64 accumulate, tiny)
    lg = logit.astype(np.float64)
    m = lg.max(axis=1, keepdims=True)
    lse = m[:, 0] + np.log(np.exp(lg - m).sum(axis=1))
    picked = lg[np.arange(B), target.astype(np.int64)]
    loss = np.float32(-(picked - lse).mean())

    if _want is not None:
        return (logit, loss), res
    return (logit, loss)
